# Initial kernel scaffold
#
"""GraphSAGE (3-layer, sum-aggregate) + mean-pool + FC + log_softmax on 8 trn2 cores.

Sharding: nodes/edges partitioned by destination node range (12500 nodes/core).
Each layer:
  1. every core computes its slice of the gather table  t_l = h_{l-1} @ Wl_l
     (rows layout) and AllGathers the full [100000,128] table to DRAM
  2. per 128-dst tile: indirect-DMA gather of source rows from the table,
     one-hot selection matrix S^T built on DVE (dst compare vs iota), and a
     chain of PE matmuls accumulating  aggT = (S @ rows)^T  plus the self term
     Wr^T @ h^T into one PSUM tile; bias+relu via ACT writes the new h^T state.
Pooling: per-128-node chunk one-hot graph matrix B (batch compare vs iota),
  h chunk PE-transposed, pool_psum[G,f] += B^T @ h_chunk; AllReduce; FC;
  log_softmax.  All cores produce identical output; core 0's is returned.
"""

import sys
import numpy as np

sys.path.insert(0, "/opt/trn_rl_repo")
sys.path.insert(0, "/opt/pypackages")

import concourse.bass as bass
import concourse.bacc as bacc
import concourse.mybir as mybir
import concourse.tile as tile
from concourse.masks import make_identity
from concourse.bass_utils import run_bass_kernel_spmd

F32 = mybir.dt.float32
I32 = mybir.dt.int32
I16 = mybir.dt.int16
BF16 = mybir.dt.bfloat16

N_NODES = 100000
N_EDGES = 1600000
F = 128          # feature dim (in = hid = 128)
OUT_DIM = 64
G = 128          # graphs
NC_CORES = 8
NPC = N_NODES // NC_CORES      # 12500 nodes per core
T = (NPC + 127) // 128         # 98 dst tiles per core
NPAD = T * 128                 # 12544 padded node columns per core
LAST_W = NPC - (T - 1) * 128   # 84 valid rows in last tile
SG = 4                         # table slices (int16 index range)
GROUP_ROWS = N_NODES // SG     # 25000 rows per slice

_CACHE = {}
_LAST_RES = None
DEBUG = False


def _build(CG: int):
    """Build the 8-core SPMD Bass program, CG gather-chunks per (tile, group)."""
    C = SG * CG                    # total chunks per dst tile
    NI = CG * 128                  # idxs per dma_gather
    nc = bacc.Bacc("TRN2", target_bir_lowering=False, debug=False,
                   num_devices=NC_CORES)

    # ---- external I/O ----
    xT = nc.dram_tensor("xT", [F, NPAD], F32, kind="ExternalInput").ap()
    idx_d = nc.dram_tensor("idx", [128, T * SG * (NI // 16)], I16,
                           kind="ExternalInput").ap()
    dstv_d = nc.dram_tensor("dstv", [128, T * C], BF16, kind="ExternalInput").ap()
    bvals_d = nc.dram_tensor("bvals", [128, T], F32, kind="ExternalInput").ap()
    recip_d = nc.dram_tensor("recip", [128, 1], F32, kind="ExternalInput").ap()
    Wl_d = [nc.dram_tensor(f"Wl{i}", [F, F], F32, kind="ExternalInput").ap()
            for i in range(3)]
    Wr_d = [nc.dram_tensor(f"Wr{i}", [F, F], F32, kind="ExternalInput").ap()
            for i in range(3)]
    bl_d = [nc.dram_tensor(f"bl{i}", [F, 1], F32, kind="ExternalInput").ap()
            for i in range(3)]
    Wfc_d = nc.dram_tensor("Wfc", [F, OUT_DIM], F32, kind="ExternalInput").ap()
    bfc_d = nc.dram_tensor("bfc", [1, OUT_DIM], F32, kind="ExternalInput").ap()
    out_d = nc.dram_tensor("out", [G, OUT_DIM], F32, kind="ExternalOutput").ap()

    # ---- internal DRAM: per-layer local slice + allgathered full table ----
    tab_loc = [nc.dram_tensor(f"tabloc{i}", [NPC, F], BF16).ap() for i in range(3)]
    tab_full = [nc.dram_tensor(f"tabfull{i}", [N_NODES, F], BF16,
                               addr_space="Shared").ap() for i in range(3)]
    pool_loc = nc.dram_tensor("poolloc", [G, F], F32).ap()
    pool_full = nc.dram_tensor("poolfull", [G, F], F32, addr_space="Shared").ap()

    if DEBUG:
        dbg_tab = [nc.dram_tensor(f"dbg_tab{i}", [N_NODES, F], F32,
                                  kind="ExternalOutput").ap() for i in range(3)]
        dbg_state = [nc.dram_tensor(f"dbg_state{i}", [F, NPAD], F32,
                                    kind="ExternalOutput").ap() for i in range(3)]
        dbg_pool = nc.dram_tensor("dbg_pool", [G, F], F32,
                                  kind="ExternalOutput").ap()
        dbg_gb = nc.dram_tensor("dbg_gb", [128, C * F], BF16,
                                kind="ExternalOutput").ap()
        dbg_st = nc.dram_tensor("dbg_st", [128, C * 128], BF16,
                                kind="ExternalOutput").ap()

    groups = [list(range(NC_CORES))]

    with tile.TileContext(nc) as tc:
        with tc.tile_pool(name="const", bufs=1) as cp:
            # persistent SBUF state
            stateT = cp.tile([F, NPAD], F32)          # h^T, f on partitions
            idx_sb = cp.tile([128, T * SG * (NI // 16)], I16)
            dstv_sb = cp.tile([128, T * C], BF16)
            bvals_sb = cp.tile([128, T], F32)
            recip_sb = cp.tile([128, 1], F32)
            iota_i = cp.tile([128, 128], I32)
            iota_f = cp.tile([128, 128], F32)
            iota_b = cp.tile([128, 128], BF16)
            ident = cp.tile([128, 128], F32)
            Wl_sb = [cp.tile([F, F], F32, name=f"wl{i}") for i in range(3)]
            Wr_sb = [cp.tile([F, F], F32, name=f"wr{i}") for i in range(3)]
            bl_sb = [cp.tile([F, 1], F32, name=f"bls{i}") for i in range(3)]
            Wfc_sb = cp.tile([F, OUT_DIM], F32)
            bfc_sb = cp.tile([1, OUT_DIM], F32)
            ones_sb = cp.tile([1, 128], F32)

            nc.sync.dma_start(out=stateT[:], in_=xT[:])
            nc.sync.dma_start(out=idx_sb[:], in_=idx_d[:])
            nc.sync.dma_start(out=dstv_sb[:], in_=dstv_d[:])
            nc.sync.dma_start(out=bvals_sb[:], in_=bvals_d[:])
            nc.sync.dma_start(out=recip_sb[:], in_=recip_d[:])
            for i in range(3):
                nc.sync.dma_start(out=Wl_sb[i][:], in_=Wl_d[i][:])
                nc.sync.dma_start(out=Wr_sb[i][:], in_=Wr_d[i][:])
                nc.sync.dma_start(out=bl_sb[i][:], in_=bl_d[i][:])
            nc.sync.dma_start(out=Wfc_sb[:], in_=Wfc_d[:])
            nc.sync.dma_start(out=bfc_sb[:], in_=bfc_d[:])
            nc.gpsimd.iota(iota_i[:], pattern=[[1, 128]], channel_multiplier=0)
            nc.vector.tensor_copy(out=iota_f[:], in_=iota_i[:])
            nc.vector.tensor_copy(out=iota_b[:], in_=iota_i[:])
            make_identity(nc, ident[:])
            nc.vector.memset(ones_sb[:], 1.0)

            with tc.tile_pool(name="work", bufs=3) as wp, \
                 tc.tile_pool(name="stw", bufs=2) as sp, \
                 tc.tile_pool(name="psA", bufs=2, space="PSUM") as psA, \
                 tc.tile_pool(name="psB", bufs=3, space="PSUM") as psB:

                for layer in range(3):
                    # --- phase 1: local table slice t_l = h @ Wl (rows) ---
                    for t in range(T):
                        cols = slice(t * 128, (t + 1) * 128)
                        pt = psA.tile([128, F], F32, tag="pA")
                        nc.tensor.matmul(out=pt[:], lhsT=stateT[:, cols],
                                         rhs=Wl_sb[layer][:],
                                         start=True, stop=True)
                        ts_sb = wp.tile([128, F], BF16, tag="tabsb")
                        nc.scalar.activation(out=ts_sb[:], in_=pt[:],
                                             func=mybir.ActivationFunctionType.Copy)
                        w = 128 if t < T - 1 else LAST_W
                        nc.sync.dma_start(out=tab_loc[layer][t * 128:t * 128 + w, :],
                                          in_=ts_sb[:w, :])

                    # --- phase 2: allgather full table ---
                    nc.gpsimd.collective_compute(
                        "AllGather", mybir.AluOpType.bypass,
                        replica_groups=groups,
                        ins=[tab_loc[layer][:]],
                        outs=[tab_full[layer][:]],
                    )

                    if DEBUG:
                        nc.sync.dma_start(out=dbg_tab[layer][:],
                                          in_=tab_full[layer][:])

                    # --- phase 3: gather + scatter-matmul + self + relu ---
                    for t in range(T):
                        cols = slice(t * 128, (t + 1) * 128)
                        gb = wp.tile([128, C, F], BF16, tag="gather")
                        for g in range(SG):
                            blk = (t * SG + g) * (NI // 16)
                            nc.gpsimd.dma_gather(
                                gb[:, g * CG:(g + 1) * CG, :],
                                tab_full[layer][g * GROUP_ROWS:(g + 1) * GROUP_ROWS, :],
                                idx_sb[:, blk:blk + NI // 16],
                                NI, NI, F,
                            )
                        # S^T [128e, C*128dst]: one DVE op via stride-0 APs
                        st = sp.tile([128, C, 128], BF16, tag="sel")
                        dsl = dstv_sb[:, t * C:(t + 1) * C]
                        d3 = bass.AP(dsl.tensor, dsl.offset,
                                     [dsl.ap[0], dsl.ap[1], [0, 128]])
                        io = iota_b[:]
                        i3 = bass.AP(io.tensor, io.offset,
                                     [io.ap[0], [0, C], io.ap[1]])
                        nc.vector.tensor_tensor(out=st[:], in0=d3, in1=i3,
                                                op=mybir.AluOpType.is_equal)
                        if DEBUG and layer == 0 and t == 0:
                            gbf = bass.AP(gb[:].tensor, gb[:].offset,
                                          [gb[:].ap[0], [1, C * F]])
                            stf = bass.AP(st[:].tensor, st[:].offset,
                                          [st[:].ap[0], [1, C * 128]])
                            nc.sync.dma_start(out=dbg_gb[:], in_=gbf)
                            nc.sync.dma_start(out=dbg_st[:], in_=stf)
                        pa = psB.tile([128, 128], F32, tag="pB")
                        for j in range(C):
                            nc.tensor.matmul(out=pa[:], lhsT=gb[:, j, :],
                                             rhs=st[:, j, :],
                                             start=(j == 0), stop=False)
                        nc.tensor.matmul(out=pa[:], lhsT=Wr_sb[layer][:],
                                         rhs=stateT[:, cols],
                                         start=False, stop=True)
                        nc.scalar.activation(out=stateT[:, cols], in_=pa[:],
                                             func=mybir.ActivationFunctionType.Relu,
                                             bias=bl_sb[layer][:])

                    if DEBUG:
                        nc.sync.dma_start(out=dbg_state[layer][:],
                                          in_=stateT[:])

                # ---- pooling: pool_psum[G,f] += B_c^T @ h_c over chunks ----
                pp = psB.tile([128, 128], F32, tag="pB")
                for t in range(T):
                    cols = slice(t * 128, (t + 1) * 128)
                    ptr = psA.tile([128, 128], F32, tag="pA")
                    nc.tensor.transpose(out=ptr[:], in_=stateT[:, cols],
                                        identity=ident[:])
                    hrow = wp.tile([128, F], F32, tag="hrow")
                    nc.scalar.activation(out=hrow[:], in_=ptr[:],
                                         func=mybir.ActivationFunctionType.Copy)
                    bc = sp.tile([128, 128], F32, tag="bonehot")
                    nc.vector.tensor_tensor(
                        out=bc[:],
                        in0=bvals_sb[:, t:t + 1].to_broadcast([128, 128]),
                        in1=iota_f[:], op=mybir.AluOpType.is_equal)
                    nc.tensor.matmul(out=pp[:], lhsT=bc[:], rhs=hrow[:],
                                     start=(t == 0), stop=(t == T - 1))

                pool_sb = wp.tile([G, F], F32)
                nc.scalar.activation(out=pool_sb[:], in_=pp[:],
                                     func=mybir.ActivationFunctionType.Copy)
                nc.sync.dma_start(out=pool_loc[:], in_=pool_sb[:])
                nc.gpsimd.collective_compute(
                    "AllReduce", mybir.AluOpType.add,
                    replica_groups=groups,
                    ins=[pool_loc[:]], outs=[pool_full[:]],
                )
                pooled = wp.tile([G, F], F32)
                nc.sync.dma_start(out=pooled[:], in_=pool_full[:])
                nc.vector.tensor_scalar_mul(pooled[:], pooled[:], recip_sb[:, :1])
                if DEBUG:
                    nc.sync.dma_start(out=dbg_pool[:], in_=pooled[:])

                # logits = pooled @ Wfc + bfc  (need pooled^T as lhsT)
                ptp = psA.tile([128, 128], F32, tag="pA")
                nc.tensor.transpose(out=ptp[:], in_=pooled[:], identity=ident[:])
                pooledT = wp.tile([F, G], F32)
                nc.scalar.activation(out=pooledT[:], in_=ptp[:],
                                     func=mybir.ActivationFunctionType.Copy)
                pl = psA.tile([128, OUT_DIM], F32, tag="pA")
                nc.tensor.matmul(out=pl[:], lhsT=pooledT[:], rhs=Wfc_sb[:],
                                 start=True, stop=False)
                nc.tensor.matmul(out=pl[:], lhsT=ones_sb[:], rhs=bfc_sb[:],
                                 start=False, stop=True)

                # log_softmax over free dim (64)
                lg = wp.tile([G, OUT_DIM], F32)
                nc.scalar.activation(out=lg[:], in_=pl[:],
                                     func=mybir.ActivationFunctionType.Copy)
                mx = wp.tile([G, 1], F32)
                nc.vector.tensor_reduce(out=mx[:], in_=lg[:],
                                        axis=mybir.AxisListType.X,
                                        op=mybir.AluOpType.max)
                sh = wp.tile([G, OUT_DIM], F32)
                nc.vector.tensor_scalar_sub(sh[:], lg[:], mx[:, :1])
                ex = wp.tile([G, OUT_DIM], F32)
                zs = wp.tile([G, 1], F32)
                nc.scalar.activation(out=ex[:], in_=sh[:],
                                     func=mybir.ActivationFunctionType.Exp,
                                     accum_out=zs[:])
                lz = wp.tile([G, 1], F32)
                nc.scalar.activation(out=lz[:], in_=zs[:],
                                     func=mybir.ActivationFunctionType.Ln)
                res = wp.tile([G, OUT_DIM], F32)
                nc.vector.tensor_scalar_sub(res[:], sh[:], lz[:, :1])
                nc.sync.dma_start(out=out_d[:], in_=res[:])

    nc.compile()
    return nc


def _prep(x, edge_index, batch):
    import ml_dtypes
    src = np.asarray(edge_index[0], dtype=np.int64)
    dst = np.asarray(edge_index[1], dtype=np.int64)
    core = dst // NPC
    nloc = dst - core * NPC
    t = nloc >> 7
    dl = nloc & 127
    grp = src // GROUP_ROWS
    seg = (core * T + t) * SG + grp            # (core, tile, group) segment id
    order = np.argsort(seg, kind="stable")
    cnt = np.bincount(seg, minlength=NC_CORES * T * SG)
    CG = int(-(-cnt.max() // 128))
    NI = CG * 128
    CC = SG * CG
    starts = np.zeros(NC_CORES * T * SG, np.int64)
    starts[1:] = np.cumsum(cnt)[:-1]
    k = np.arange(N_EDGES) - starts[seg[order]]
    p = k & 127
    jl = k >> 7
    oc = core[order]
    ot = t[order]
    og = grp[order]
    j = og * CG + jl
    dstv = np.full((NC_CORES, 128, T * CC), -1.0, np.float32)
    dstv[oc, p, ot * CC + j] = dl[order].astype(np.float32)
    # int16 relative indices in dma_gather wrapped layout [16, NI/16] per seg
    idx16 = np.zeros((NC_CORES, T * SG, NI), np.int16)
    idx16[oc, ot * SG + og, k] = (src[order] - og * GROUP_ROWS).astype(np.int16)
    # wrap: element i at [i%16, i//16], then replicate to 128 partitions
    idx16 = idx16.reshape(NC_CORES, T * SG, NI // 16, 16).transpose(0, 3, 1, 2)
    idx16 = idx16.reshape(NC_CORES, 16, T * SG * (NI // 16))
    idx16 = np.tile(idx16, (1, 8, 1))          # [NC, 128, T*SG*NI/16]

    loc = np.arange(NPAD)
    bvals = np.empty((NC_CORES, 128, T), np.float32)
    xT = np.zeros((NC_CORES, F, NPAD), np.float32)
    for i in range(NC_CORES):
        gid = np.minimum(i * NPC + loc, N_NODES - 1)
        bv = np.where(loc < NPC, np.asarray(batch, np.int64)[gid], -1)
        bvals[i] = bv.reshape(T, 128).T.astype(np.float32)
        xT[i, :, :NPC] = np.asarray(x, np.float32)[i * NPC:(i + 1) * NPC].T

    counts = np.bincount(np.asarray(batch, np.int64), minlength=G).astype(np.float32)
    recip = (1.0 / np.maximum(counts, 1.0)).reshape(G, 1)
    dstv16 = dstv.astype(ml_dtypes.bfloat16)
    return CG, idx16, dstv16, bvals, xT, recip


def kernel(x, edge_index, batch,
           Wl0, bl0, Wr0, Wl1, bl1, Wr1, Wl2, bl2, Wr2, Wfc, bfc,
           _want_nc=False, _trace=False, _tmpdir=None):
    CG, idx_all, dstv, bvals, xT, recip = _prep(x, edge_index, batch)
    if CG not in _CACHE:
        _CACHE[CG] = _build(CG)
    nc = _CACHE[CG]

    Wls = [np.asarray(w, np.float32) for w in (Wl0, Wl1, Wl2)]
    Wrs = [np.asarray(w, np.float32) for w in (Wr0, Wr1, Wr2)]
    bls = [np.asarray(b, np.float32).reshape(F, 1) for b in (bl0, bl1, bl2)]
    in_maps = []
    for i in range(NC_CORES):
        m = {"xT": xT[i], "idx": idx_all[i], "dstv": dstv[i],
             "bvals": bvals[i], "recip": recip,
             "Wfc": np.asarray(Wfc, np.float32),
             "bfc": np.asarray(bfc, np.float32).reshape(1, OUT_DIM)}
        for l in range(3):
            m[f"Wl{l}"] = Wls[l]
            m[f"Wr{l}"] = Wrs[l]
            m[f"bl{l}"] = bls[l]
        in_maps.append(m)

    res = run_bass_kernel_spmd(nc, in_maps, list(range(NC_CORES)),
                               trace=_trace, tmpdir=_tmpdir)
    global _LAST_RES
    _LAST_RES = res
    out = res.results[0]["out"]
    if _want_nc:
        return out, nc, in_maps
    return np.asarray(out, np.float32)



# revision 1
# speedup vs baseline: 2.9360x; 2.9360x over previous
"""GraphSAGE (3-layer, sum-aggregate) + mean-pool + FC + log_softmax on 8 trn2 cores.

Sharding: nodes/edges partitioned by destination node range (12500 nodes/core).
Each layer:
  1. every core computes its slice of the gather table  t_l = h_{l-1} @ Wl_l
     (rows layout) and AllGathers the full [100000,128] table to DRAM
  2. per 128-dst tile: indirect-DMA gather of source rows from the table,
     one-hot selection matrix S^T built on DVE (dst compare vs iota), and a
     chain of PE matmuls accumulating  aggT = (S @ rows)^T  plus the self term
     Wr^T @ h^T into one PSUM tile; bias+relu via ACT writes the new h^T state.
Pooling: per-128-node chunk one-hot graph matrix B (batch compare vs iota),
  h chunk PE-transposed, pool_psum[G,f] += B^T @ h_chunk; AllReduce; FC;
  log_softmax.  All cores produce identical output; core 0's is returned.
"""

import sys
import numpy as np

sys.path.insert(0, "/opt/trn_rl_repo")
sys.path.insert(0, "/opt/pypackages")

import concourse.bass as bass
import concourse.bacc as bacc
import concourse.mybir as mybir
import concourse.tile as tile
from concourse.masks import make_identity
from concourse.bass_utils import run_bass_kernel_spmd

F32 = mybir.dt.float32
I32 = mybir.dt.int32
I16 = mybir.dt.int16
BF16 = mybir.dt.bfloat16

N_NODES = 100000
N_EDGES = 1600000
F = 128          # feature dim (in = hid = 128)
OUT_DIM = 64
G = 128          # graphs
NC_CORES = 8
NPC = N_NODES // NC_CORES      # 12500 nodes per core
T = (NPC + 127) // 128         # 98 dst tiles per core
NPAD = T * 128                 # 12544 padded node columns per core
LAST_W = NPC - (T - 1) * 128   # 84 valid rows in last tile
SG = 4                         # table slices (int16 index range)
GROUP_ROWS = N_NODES // SG     # 25000 rows per slice

_CACHE = {}
_LAST_RES = None
DEBUG = False


def _build(CG: int):
    """Build the 8-core SPMD Bass program, CG gather-chunks per (tile, group)."""
    C = SG * CG                    # total chunks per dst tile
    NI = CG * 128                  # idxs per dma_gather
    nc = bacc.Bacc("TRN2", target_bir_lowering=False, debug=False,
                   num_devices=NC_CORES)

    # ---- external I/O ----
    xT = nc.dram_tensor("xT", [F, NPAD], F32, kind="ExternalInput").ap()
    idx_d = nc.dram_tensor("idx", [128, T * SG * (NI // 16)], I16,
                           kind="ExternalInput").ap()
    dstv_d = nc.dram_tensor("dstv", [128, T * C], BF16, kind="ExternalInput").ap()
    bvals_d = nc.dram_tensor("bvals", [128, T], F32, kind="ExternalInput").ap()
    recip_d = nc.dram_tensor("recip", [128, 1], F32, kind="ExternalInput").ap()
    Wl_d = [nc.dram_tensor(f"Wl{i}", [F, F], F32, kind="ExternalInput").ap()
            for i in range(3)]
    Wr_d = [nc.dram_tensor(f"Wr{i}", [F, F], F32, kind="ExternalInput").ap()
            for i in range(3)]
    bl_d = [nc.dram_tensor(f"bl{i}", [F, 1], F32, kind="ExternalInput").ap()
            for i in range(3)]
    Wfc_d = nc.dram_tensor("Wfc", [F, OUT_DIM], F32, kind="ExternalInput").ap()
    bfc_d = nc.dram_tensor("bfc", [1, OUT_DIM], F32, kind="ExternalInput").ap()
    out_d = nc.dram_tensor("out", [G, OUT_DIM], F32, kind="ExternalOutput").ap()

    # ---- internal DRAM: per-layer local slice + allgathered full table ----
    tab_loc = [nc.dram_tensor(f"tabloc{i}", [NPC, F], BF16).ap() for i in range(3)]
    tab_full = [nc.dram_tensor(f"tabfull{i}", [N_NODES, F], BF16,
                               addr_space="Shared").ap() for i in range(3)]
    pool_loc = nc.dram_tensor("poolloc", [G, F], F32).ap()
    pool_full = nc.dram_tensor("poolfull", [G, F], F32, addr_space="Shared").ap()

    if DEBUG:
        dbg_tab = [nc.dram_tensor(f"dbg_tab{i}", [N_NODES, F], F32,
                                  kind="ExternalOutput").ap() for i in range(3)]
        dbg_state = [nc.dram_tensor(f"dbg_state{i}", [F, NPAD], F32,
                                    kind="ExternalOutput").ap() for i in range(3)]
        dbg_pool = nc.dram_tensor("dbg_pool", [G, F], F32,
                                  kind="ExternalOutput").ap()
        dbg_gb = nc.dram_tensor("dbg_gb", [128, C * F], BF16,
                                kind="ExternalOutput").ap()
        dbg_st = nc.dram_tensor("dbg_st", [128, C * 128], BF16,
                                kind="ExternalOutput").ap()

    groups = [list(range(NC_CORES))]

    with tile.TileContext(nc) as tc:
        with tc.tile_pool(name="const", bufs=1) as cp:
            # persistent SBUF state
            stateT = cp.tile([F, NPAD], F32)          # h^T, f on partitions
            idx_sb = cp.tile([128, T * SG * (NI // 16)], I16)
            dstv_sb = cp.tile([128, T * C], BF16)
            bvals_sb = cp.tile([128, T], F32)
            recip_sb = cp.tile([128, 1], F32)
            iota_i = cp.tile([128, 128], I32)
            iota_f = cp.tile([128, 128], F32)
            iota_b = cp.tile([128, 128], BF16)
            ident = cp.tile([128, 128], F32)
            Wl_sb = [cp.tile([F, F], F32, name=f"wl{i}") for i in range(3)]
            Wr_sb = [cp.tile([F, F], F32, name=f"wr{i}") for i in range(3)]
            bl_sb = [cp.tile([F, 1], F32, name=f"bls{i}") for i in range(3)]
            Wfc_sb = cp.tile([F, OUT_DIM], F32)
            bfc_sb = cp.tile([1, OUT_DIM], F32)
            ones_sb = cp.tile([1, 128], F32)

            nc.sync.dma_start(out=stateT[:], in_=xT[:])
            nc.sync.dma_start(out=idx_sb[:], in_=idx_d[:])
            nc.sync.dma_start(out=dstv_sb[:], in_=dstv_d[:])
            nc.sync.dma_start(out=bvals_sb[:], in_=bvals_d[:])
            nc.sync.dma_start(out=recip_sb[:], in_=recip_d[:])
            for i in range(3):
                nc.sync.dma_start(out=Wl_sb[i][:], in_=Wl_d[i][:])
                nc.sync.dma_start(out=Wr_sb[i][:], in_=Wr_d[i][:])
                nc.sync.dma_start(out=bl_sb[i][:], in_=bl_d[i][:])
            nc.sync.dma_start(out=Wfc_sb[:], in_=Wfc_d[:])
            nc.sync.dma_start(out=bfc_sb[:], in_=bfc_d[:])
            nc.gpsimd.iota(iota_i[:], pattern=[[1, 128]], channel_multiplier=0)
            nc.vector.tensor_copy(out=iota_f[:], in_=iota_i[:])
            nc.vector.tensor_copy(out=iota_b[:], in_=iota_i[:])
            make_identity(nc, ident[:])
            nc.vector.memset(ones_sb[:], 1.0)

            with tc.tile_pool(name="work", bufs=3) as wp, \
                 tc.tile_pool(name="stw", bufs=2) as sp, \
                 tc.tile_pool(name="psA", bufs=2, space="PSUM") as psA, \
                 tc.tile_pool(name="psB", bufs=3, space="PSUM") as psB:

                for layer in range(3):
                    # --- phase 1: local table slice t_l = h @ Wl (rows) ---
                    for t in range(T):
                        cols = slice(t * 128, (t + 1) * 128)
                        pt = psA.tile([128, F], F32, tag="pA")
                        nc.tensor.matmul(out=pt[:], lhsT=stateT[:, cols],
                                         rhs=Wl_sb[layer][:],
                                         start=True, stop=True)
                        ts_sb = wp.tile([128, F], BF16, tag="tabsb")
                        nc.scalar.activation(out=ts_sb[:], in_=pt[:],
                                             func=mybir.ActivationFunctionType.Copy)
                        w = 128 if t < T - 1 else LAST_W
                        nc.sync.dma_start(out=tab_loc[layer][t * 128:t * 128 + w, :],
                                          in_=ts_sb[:w, :])

                    # --- phase 2: allgather full table ---
                    nc.gpsimd.collective_compute(
                        "AllGather", mybir.AluOpType.bypass,
                        replica_groups=groups,
                        ins=[tab_loc[layer][:]],
                        outs=[tab_full[layer][:]],
                    )

                    if DEBUG:
                        nc.sync.dma_start(out=dbg_tab[layer][:],
                                          in_=tab_full[layer][:])

                    # --- phase 3: gather + scatter-matmul + self + relu ---
                    for t in range(T):
                        cols = slice(t * 128, (t + 1) * 128)
                        gb = wp.tile([128, C, F], BF16, tag="gather")
                        for g in range(SG):
                            blk = (t * SG + g) * (NI // 16)
                            nc.gpsimd.dma_gather(
                                gb[:, g * CG:(g + 1) * CG, :],
                                tab_full[layer][g * GROUP_ROWS:(g + 1) * GROUP_ROWS, :],
                                idx_sb[:, blk:blk + NI // 16],
                                NI, NI, F,
                            )
                        # S^T [128e, C*128dst]: one DVE op via stride-0 APs
                        st = sp.tile([128, C, 128], BF16, tag="sel")
                        dsl = dstv_sb[:, t * C:(t + 1) * C]
                        d3 = bass.AP(dsl.tensor, dsl.offset,
                                     [dsl.ap[0], dsl.ap[1], [0, 128]])
                        io = iota_b[:]
                        i3 = bass.AP(io.tensor, io.offset,
                                     [io.ap[0], [0, C], io.ap[1]])
                        nc.vector.tensor_tensor(out=st[:], in0=d3, in1=i3,
                                                op=mybir.AluOpType.is_equal)
                        if DEBUG and layer == 0 and t == 0:
                            gbf = bass.AP(gb[:].tensor, gb[:].offset,
                                          [gb[:].ap[0], [1, C * F]])
                            stf = bass.AP(st[:].tensor, st[:].offset,
                                          [st[:].ap[0], [1, C * 128]])
                            nc.sync.dma_start(out=dbg_gb[:], in_=gbf)
                            nc.sync.dma_start(out=dbg_st[:], in_=stf)
                        pa = psB.tile([128, 128], F32, tag="pB")
                        for j in range(C):
                            nc.tensor.matmul(out=pa[:], lhsT=gb[:, j, :],
                                             rhs=st[:, j, :],
                                             start=(j == 0), stop=False)
                        nc.tensor.matmul(out=pa[:], lhsT=Wr_sb[layer][:],
                                         rhs=stateT[:, cols],
                                         start=False, stop=True)
                        nc.scalar.activation(out=stateT[:, cols], in_=pa[:],
                                             func=mybir.ActivationFunctionType.Relu,
                                             bias=bl_sb[layer][:])

                    if DEBUG:
                        nc.sync.dma_start(out=dbg_state[layer][:],
                                          in_=stateT[:])

                # ---- pooling: pool_psum[G,f] += B_c^T @ h_c over chunks ----
                pp = psB.tile([128, 128], F32, tag="pB")
                for t in range(T):
                    cols = slice(t * 128, (t + 1) * 128)
                    ptr = psA.tile([128, 128], F32, tag="pA")
                    nc.tensor.transpose(out=ptr[:], in_=stateT[:, cols],
                                        identity=ident[:])
                    hrow = wp.tile([128, F], F32, tag="hrow")
                    nc.scalar.activation(out=hrow[:], in_=ptr[:],
                                         func=mybir.ActivationFunctionType.Copy)
                    bc = sp.tile([128, 128], F32, tag="bonehot")
                    nc.vector.tensor_tensor(
                        out=bc[:],
                        in0=bvals_sb[:, t:t + 1].to_broadcast([128, 128]),
                        in1=iota_f[:], op=mybir.AluOpType.is_equal)
                    nc.tensor.matmul(out=pp[:], lhsT=bc[:], rhs=hrow[:],
                                     start=(t == 0), stop=(t == T - 1))

                pool_sb = wp.tile([G, F], F32)
                nc.scalar.activation(out=pool_sb[:], in_=pp[:],
                                     func=mybir.ActivationFunctionType.Copy)
                nc.sync.dma_start(out=pool_loc[:], in_=pool_sb[:])
                nc.gpsimd.collective_compute(
                    "AllReduce", mybir.AluOpType.add,
                    replica_groups=groups,
                    ins=[pool_loc[:]], outs=[pool_full[:]],
                )
                pooled = wp.tile([G, F], F32)
                nc.sync.dma_start(out=pooled[:], in_=pool_full[:])
                nc.vector.tensor_scalar_mul(pooled[:], pooled[:], recip_sb[:, :1])
                if DEBUG:
                    nc.sync.dma_start(out=dbg_pool[:], in_=pooled[:])

                # logits = pooled @ Wfc + bfc  (need pooled^T as lhsT)
                ptp = psA.tile([128, 128], F32, tag="pA")
                nc.tensor.transpose(out=ptp[:], in_=pooled[:], identity=ident[:])
                pooledT = wp.tile([F, G], F32)
                nc.scalar.activation(out=pooledT[:], in_=ptp[:],
                                     func=mybir.ActivationFunctionType.Copy)
                pl = psA.tile([128, OUT_DIM], F32, tag="pA")
                nc.tensor.matmul(out=pl[:], lhsT=pooledT[:], rhs=Wfc_sb[:],
                                 start=True, stop=False)
                nc.tensor.matmul(out=pl[:], lhsT=ones_sb[:], rhs=bfc_sb[:],
                                 start=False, stop=True)

                # log_softmax over free dim (64)
                lg = wp.tile([G, OUT_DIM], F32)
                nc.scalar.activation(out=lg[:], in_=pl[:],
                                     func=mybir.ActivationFunctionType.Copy)
                mx = wp.tile([G, 1], F32)
                nc.vector.tensor_reduce(out=mx[:], in_=lg[:],
                                        axis=mybir.AxisListType.X,
                                        op=mybir.AluOpType.max)
                sh = wp.tile([G, OUT_DIM], F32)
                nc.vector.tensor_scalar_sub(sh[:], lg[:], mx[:, :1])
                ex = wp.tile([G, OUT_DIM], F32)
                zs = wp.tile([G, 1], F32)
                nc.scalar.activation(out=ex[:], in_=sh[:],
                                     func=mybir.ActivationFunctionType.Exp,
                                     accum_out=zs[:])
                lz = wp.tile([G, 1], F32)
                nc.scalar.activation(out=lz[:], in_=zs[:],
                                     func=mybir.ActivationFunctionType.Ln)
                res = wp.tile([G, OUT_DIM], F32)
                nc.vector.tensor_scalar_sub(res[:], sh[:], lz[:, :1])
                nc.sync.dma_start(out=out_d[:], in_=res[:])

    nc.compile()
    return nc


def _prep(x, edge_index, batch):
    import ml_dtypes
    src = np.asarray(edge_index[0], dtype=np.int64)
    dst = np.asarray(edge_index[1], dtype=np.int64)
    core = dst // NPC
    nloc = dst - core * NPC
    t = nloc >> 7
    dl = nloc & 127
    grp = src // GROUP_ROWS
    seg = (core * T + t) * SG + grp            # (core, tile, group) segment id
    order = np.argsort(seg, kind="stable")
    cnt = np.bincount(seg, minlength=NC_CORES * T * SG)
    CG = int(-(-cnt.max() // 128))
    NI = CG * 128
    CC = SG * CG
    starts = np.zeros(NC_CORES * T * SG, np.int64)
    starts[1:] = np.cumsum(cnt)[:-1]
    k = np.arange(N_EDGES) - starts[seg[order]]
    p = k & 127
    jl = k >> 7
    oc = core[order]
    ot = t[order]
    og = grp[order]
    j = og * CG + jl
    dstv = np.full((NC_CORES, 128, T * CC), -1.0, np.float32)
    dstv[oc, p, ot * CC + j] = dl[order].astype(np.float32)
    # int16 relative indices in dma_gather wrapped layout [16, NI/16] per seg
    idx16 = np.zeros((NC_CORES, T * SG, NI), np.int16)
    idx16[oc, ot * SG + og, k] = (src[order] - og * GROUP_ROWS).astype(np.int16)
    # wrap: element i at [i%16, i//16], then replicate to 128 partitions
    idx16 = idx16.reshape(NC_CORES, T * SG, NI // 16, 16).transpose(0, 3, 1, 2)
    idx16 = idx16.reshape(NC_CORES, 16, T * SG * (NI // 16))
    idx16 = np.tile(idx16, (1, 8, 1))          # [NC, 128, T*SG*NI/16]

    loc = np.arange(NPAD)
    bvals = np.empty((NC_CORES, 128, T), np.float32)
    xT = np.zeros((NC_CORES, F, NPAD), np.float32)
    for i in range(NC_CORES):
        gid = np.minimum(i * NPC + loc, N_NODES - 1)
        bv = np.where(loc < NPC, np.asarray(batch, np.int64)[gid], -1)
        bvals[i] = bv.reshape(T, 128).T.astype(np.float32)
        xT[i, :, :NPC] = np.asarray(x, np.float32)[i * NPC:(i + 1) * NPC].T

    counts = np.bincount(np.asarray(batch, np.int64), minlength=G).astype(np.float32)
    recip = (1.0 / np.maximum(counts, 1.0)).reshape(G, 1)
    dstv16 = dstv.astype(ml_dtypes.bfloat16)
    return CG, idx16, dstv16, bvals, xT, recip


def kernel(x, edge_index, batch,
           Wl0, bl0, Wr0, Wl1, bl1, Wr1, Wl2, bl2, Wr2, Wfc, bfc,
           _want_nc=False, _trace=False, _tmpdir=None):
    CG, idx_all, dstv, bvals, xT, recip = _prep(x, edge_index, batch)
    if CG not in _CACHE:
        _CACHE[CG] = _build(CG)
    nc = _CACHE[CG]

    Wls = [np.asarray(w, np.float32) for w in (Wl0, Wl1, Wl2)]
    Wrs = [np.asarray(w, np.float32) for w in (Wr0, Wr1, Wr2)]
    bls = [np.asarray(b, np.float32).reshape(F, 1) for b in (bl0, bl1, bl2)]
    in_maps = []
    for i in range(NC_CORES):
        m = {"xT": xT[i], "idx": idx_all[i], "dstv": dstv[i],
             "bvals": bvals[i], "recip": recip,
             "Wfc": np.asarray(Wfc, np.float32),
             "bfc": np.asarray(bfc, np.float32).reshape(1, OUT_DIM)}
        for l in range(3):
            m[f"Wl{l}"] = Wls[l]
            m[f"Wr{l}"] = Wrs[l]
            m[f"bl{l}"] = bls[l]
        in_maps.append(m)

    res = run_bass_kernel_spmd(nc, in_maps, list(range(NC_CORES)),
                               trace=_trace, tmpdir=_tmpdir)
    global _LAST_RES
    _LAST_RES = res
    out = res.results[0]["out"]
    if _want_nc:
        return out, nc, in_maps
    return np.asarray(out, np.float32)

